# revision 11
# baseline (speedup 1.0000x reference)
"""Trainium2 Bass kernel for nn_AttnDecoderRNN2 (single decoder step, eval mode).

Math: the previous-attention state is a one-hot at t=0 (null_state branch), so
the windowed mask keeps only positions t in [0, ATT_RANGE), and the global max
subtraction cancels exactly in the L1 normalization.  The [N, T, ATT]
attention-bias tensor therefore only matters on a 10-wide window.  The zero
initial LSTM state kills the Whh terms and the f gate, so only the i/g/o
columns of Wih0/Wih1 are shipped.

Layout: the big LSTM/output matmuls run "flipped" — activations are the
stationary operand ([K, batch] column tiles), weight chunks are the moving
operand streamed straight from HBM — so the PE never pays fp32 LDWEIGHTS on
the 24 MB of weights and compute overlaps the weight DMA chunk-by-chunk.
Biases ride along as ones-rows appended to the stationary operand with the
bias vectors appended as extra weight rows.

Sharding: pure data-parallel over batch N=64 across 8 cores (8 batches/core),
weights replicated, each packed host-side into a handful of large DMA-friendly
tensors.
"""

import numpy as np

import concourse.bass as bass
import concourse.mybir as mybir
import concourse.tile as tile
from concourse import bacc
from concourse.bass_utils import run_bass_kernel_spmd
from concourse.masks import make_identity

AF = mybir.ActivationFunctionType
ALU = mybir.AluOpType
FP32 = mybir.dt.float32
FP16 = mybir.dt.float16
LSTM_DT = FP16          # dtype of the streamed LSTM weight chunks + stationaries

N_CORES = 8
N = 64
ENC, ATT, DEC, SPK, O, R = 512, 256, 256, 64, 80, 2
H4 = 4 * DEC                 # 1024
G3 = 3 * H4                  # 3072 gates kept (i, g, o; f is dead)
B = N // N_CORES             # batches per core = 8
W = 10                       # attention window = ATT_RANGE
BW = B * W                   # 80

# mat_a column offsets (attention weights, loaded first)
MP_WENC = 0            # 4 blocks x 256
MP_WSPKR = 1024        # [0:64] x 256
MPA_COLS = 1280
# mat_b column offsets (prenet / speed-proj weights)
MP_WP1A = 0            # [128] x 512
MP_WP1B = 512          # [0:16] x 512
MP_WP2 = 1024          # 4 blocks x 256
MP_WSP2 = 2048         # 2 blocks x 512
MPB_COLS = 3072

# small_cols column offsets
SC_BENC, SC_WSPEED, SC_WPROJ, SC_BP1, SC_BP2, SC_WSP1, SC_BSP1, SC_BSP2, \
    SC_BPROJ = 0, 2, 4, 6, 10, 12, 14, 16, 20
SC_COLS = 21

# core_pack column offsets
CP_XE = 0              # 4 blocks x 80
CP_XD0 = 320           # [128] x 8
CP_XD1 = 328           # [0:16] x 8
CP_SPKA = 336          # [0:66] x 8 (rows 64:66 = 1.0 bias-ones)
CP_COLS = 344

_CACHE: dict = {}


def _build_nc(loop_n=None):
    nc = bacc.Bacc(None)

    def inp(name, shape):
        return nc.dram_tensor(name, list(shape), FP32, kind="ExternalInput")

    d = {
        "core_pack": inp("core_pack", [128, CP_COLS]),
        "rowvec": inp("rowvec", [1, B + W]),
        "speed_row": inp("speed_row", [1, B]),
        "mat_a": inp("mat_a", [128, MPA_COLS]),
        "mat_b": inp("mat_b", [128, MPB_COLS]),
        "small_cols": inp("small_cols", [128, SC_COLS]),
        "prev_pack": inp("prev_pack", [128, 2 * W]),
        "w0_pack": nc.dram_tensor("w0_pack", [ENC + DEC + SPK + 2, G3], LSTM_DT,
                                  kind="ExternalInput"),
        "w1_pack": nc.dram_tensor("w1_pack", [H4 + 2, G3], LSTM_DT,
                                  kind="ExternalInput"),
        "wout_pack": inp("wout_pack", [128, 13 * O * R]),
    }
    y_out = nc.dram_tensor("y_out", [B, O * R], FP32, kind="ExternalOutput")
    y_ctx = nc.dram_tensor("y_ctx", [ENC, B], FP32, kind="ExternalOutput")

    W0_CHUNKS = [(k * 128, 128) for k in range(6)] + [(768, 66)]
    W1_CHUNKS = [(k * 128, 128) for k in range(8)] + [(1024, 2)]

    with tile.TileContext(nc) as tc:
        with (
            tc.tile_pool(name="const", bufs=1 if loop_n is None else 2) as PC,
            tc.tile_pool(name="wbig", bufs=9) as PW,
            tc.tile_pool(name="scr", bufs=2) as PS,
            tc.tile_pool(name="ps", bufs=2, space="PSUM") as PP,
            tc.tile_pool(name="pg", bufs=1, space="PSUM") as PPG,
        ):
            def body():
                # ---- bulk constant loads ----
                cp = PC.tile([128, CP_COLS], FP32, tag="cp")
                nc.sync.dma_start(out=cp[:], in_=d["core_pack"][:])
                rv = PC.tile([1, B + W], FP32, tag="rv")
                nc.sync.dma_start(out=rv[:], in_=d["rowvec"][:])
                speed_bc = PC.tile([128, B], FP32, tag="speed_bc")
                nc.sync.dma_start(out=speed_bc[:],
                                  in_=d["speed_row"][:].to_broadcast([128, B]))
                mata = PC.tile([128, MPA_COLS], FP32, tag="mata")
                nc.sync.dma_start(out=mata[:], in_=d["mat_a"][:])
                sc = PC.tile([128, SC_COLS], FP32, tag="sc")
                nc.sync.dma_start(out=sc[:], in_=d["small_cols"][:])
                prev = PC.tile([128, 2 * W], FP32, tag="prev")
                nc.sync.dma_start(out=prev[:], in_=d["prev_pack"][:])
                matb = PC.tile([128, MPB_COLS], FP32, tag="matb")
                nc.sync.dma_start(out=matb[:], in_=d["mat_b"][:])
                ident = PC.tile([B, B], FP32, tag="ident")
                make_identity(nc, ident[:])
                ones1 = PC.tile([1, B], FP32, tag="ones1")
                nc.vector.memset(ones1[:], 1.0)
                ones2 = PC.tile([2, B], LSTM_DT, tag="ones2")
                nc.vector.memset(ones2[:], 1.0)
                onesrep = PC.tile([1, 128], FP32, tag="onesrep")
                nc.vector.memset(onesrep[:], 1.0)

                xe = [cp[:, CP_XE + k * BW:CP_XE + (k + 1) * BW] for k in range(4)]
                len1 = rv[:, 0:B]
                t_row = rv[:, B:B + W]

                def softsign(tagbase, src_ap, p, f):
                    ax = PS.tile([p, f], FP32, tag=tagbase + "_a")
                    nc.scalar.activation(ax[:], src_ap, AF.Abs)
                    den = PS.tile([p, f], FP32, tag=tagbase + "_d")
                    nc.vector.tensor_scalar_add(den[:], ax[:], 1.0)
                    rec = PS.tile([p, f], FP32, tag=tagbase + "_r")
                    nc.vector.reciprocal(rec[:], den[:])
                    out = PS.tile([p, f], FP32, tag=tagbase)
                    nc.vector.tensor_mul(out[:], src_ap, rec[:])
                    return out

                # ---- attention bias from encoder window ----
                ab = []
                for m in range(2):
                    pe = PP.tile([128, BW], FP32, tag="ps")
                    for k in range(4):
                        nc.tensor.matmul(
                            pe[:],
                            mata[:, MP_WENC + k * 256 + m * 128:
                                 MP_WENC + k * 256 + (m + 1) * 128],
                            xe[k], start=(k == 0), stop=(k == 3))
                    eb = PS.tile([128, BW], FP32, tag=f"eb{m}")
                    nc.vector.tensor_scalar_add(eb[:], pe[:],
                                                sc[:, SC_BENC + m:SC_BENC + m + 1])
                    ab.append(softsign(f"ab{m}", eb[:], 128, BW))

                # ---- speaker bias ----
                sb = []
                for m in range(2):
                    pk = PP.tile([128, B], FP32, tag="ps")
                    nc.tensor.matmul(
                        pk[:],
                        mata[0:SPK, MP_WSPKR + m * 128:MP_WSPKR + (m + 1) * 128],
                        cp[0:SPK, CP_SPKA:CP_SPKA + B], start=True, stop=True)
                    sb.append(softsign(f"sb{m}", pk[:], 128, B))

                # ---- bias_an[a, n] = spkr_bias + speed * wspeed ----
                bias_an = []
                for m in range(2):
                    scv = PS.tile([128, B], FP32, tag=f"spc{m}")
                    nc.vector.tensor_scalar_mul(
                        scv[:], speed_bc[:], sc[:, SC_WSPEED + m:SC_WSPEED + m + 1])
                    bn = PS.tile([128, B], FP32, tag=f"ban{m}")
                    nc.vector.tensor_add(bn[:], sb[m][:], scv[:])
                    bias_an.append(bn)

                # ---- e = ab + bias_an (bcast t) + prev (bcast n); tanh ----
                th = []
                for m in range(2):
                    e3 = PS.tile([128, B, W], FP32, tag=f"e3{m}")
                    ab3 = ab[m][:].rearrange("p (n w) -> p n w", w=W)
                    bn3 = bias_an[m][:].unsqueeze(2).to_broadcast([128, B, W])
                    nc.vector.tensor_add(e3[:], ab3, bn3)
                    pv3 = prev[:, m * W:(m + 1) * W].unsqueeze(1) \
                        .to_broadcast([128, B, W])
                    nc.vector.tensor_add(e3[:], e3[:], pv3)
                    t3 = PS.tile([128, BW], FP32, tag=f"th{m}")
                    nc.scalar.activation(t3[:],
                                         e3[:].rearrange("p n w -> p (n w)"),
                                         AF.Tanh)
                    th.append(t3)

                # ---- logit = tanh(e) . wproj + bproj ----
                pl = PP.tile([1, BW], FP32, tag="ps")
                for m in range(2):
                    nc.tensor.matmul(pl[:], sc[:, SC_WPROJ + m:SC_WPROJ + m + 1],
                                     th[m][:], start=(m == 0), stop=(m == 1))
                logit = PS.tile([1, BW], FP32, tag="logit")
                nc.vector.tensor_scalar_add(logit[:], pl[:],
                                            sc[0:1, SC_BPROJ:SC_BPROJ + 1])

                # ---- windowed masked softmax (L1-normalized) ----
                lg3 = logit[:].rearrange("p (n w) -> p n w", w=W)
                mx = PS.tile([1, B], FP32, tag="mx")
                nc.vector.tensor_reduce(mx[:], lg3, mybir.AxisListType.X, ALU.max)
                sh = PS.tile([1, B, W], FP32, tag="sh")
                nc.vector.tensor_tensor(sh[:], lg3,
                                        mx[:].unsqueeze(2).to_broadcast([1, B, W]),
                                        ALU.subtract)
                ex = PS.tile([1, B, W], FP32, tag="ex")
                nc.scalar.activation(ex[:], sh[:], AF.Exp)
                mask = PS.tile([1, B, W], FP32, tag="mask")
                nc.vector.tensor_tensor(
                    mask[:], t_row.unsqueeze(1).to_broadcast([1, B, W]),
                    len1.unsqueeze(2).to_broadcast([1, B, W]), ALU.is_le)
                wm = PS.tile([1, B, W], FP32, tag="wm")
                nc.vector.tensor_mul(wm[:], ex[:], mask[:])
                den = PS.tile([1, B], FP32, tag="den")
                nc.vector.tensor_reduce(den[:], wm[:], mybir.AxisListType.X, ALU.add)
                nc.vector.tensor_scalar_max(den[:], den[:], 1e-12)
                rec = PS.tile([1, B], FP32, tag="recd")
                nc.vector.reciprocal(rec[:], den[:])
                att = PS.tile([1, B, W], FP32, tag="att")
                nc.vector.tensor_tensor(att[:], wm[:],
                                        rec[:].unsqueeze(2).to_broadcast([1, B, W]),
                                        ALU.mult)

                # ---- replicate att across 128 partitions via K=1 matmul ----
                pr = PP.tile([128, BW], FP32, tag="ps")
                nc.tensor.matmul(pr[:], onesrep[:],
                                 att[:].rearrange("p n w -> p (n w)"),
                                 start=True, stop=True)
                att_rep = PS.tile([128, BW], FP32, tag="attrep")
                nc.vector.tensor_copy(att_rep[:], pr[:])

                # ---- speed projection sp = tanh(relu(speed*w+b) @ Wsp2T + b2) ----
                r1 = []
                for m in range(2):
                    r = PS.tile([128, B], FP32, tag=f"r1{m}")
                    nc.vector.tensor_scalar(r[:], speed_bc[:],
                                            sc[:, SC_WSP1 + m:SC_WSP1 + m + 1],
                                            sc[:, SC_BSP1 + m:SC_BSP1 + m + 1],
                                            ALU.mult, ALU.add)
                    nc.scalar.activation(r[:], r[:], AF.Relu)
                    r1.append(r)
                sp_sb = []
                for m3 in range(4):
                    psp = PP.tile([128, B], FP32, tag="ps")
                    for k in range(2):
                        nc.tensor.matmul(
                            psp[:],
                            matb[:, MP_WSP2 + k * 512 + m3 * 128:
                                 MP_WSP2 + k * 512 + (m3 + 1) * 128],
                            r1[k][:], start=(k == 0), stop=(k == 1))
                    s = PS.tile([128, B], FP32, tag=f"sp{m3}")
                    nc.scalar.activation(s[:], psp[:], AF.Tanh,
                                         bias=sc[:, SC_BSP2 + m3:SC_BSP2 + m3 + 1])
                    sp_sb.append(s)

                # ---- context[e, n] = sum_t att[n,t] * (xe + sp)[e, n, t] ----
                ctx = []
                for k in range(4):
                    em = PS.tile([128, B, W], FP32, tag=f"em{k}")
                    nc.vector.tensor_add(
                        em[:], xe[k].rearrange("p (n w) -> p n w", w=W),
                        sp_sb[k][:].unsqueeze(2).to_broadcast([128, B, W]))
                    nc.vector.tensor_mul(
                        em[:], em[:], att_rep[:].rearrange("p (n w) -> p n w", w=W))
                    c = PC.tile([128, B], FP32, tag=f"ctx{k}")
                    nc.vector.tensor_reduce(c[:], em[:], mybir.AxisListType.X,
                                            ALU.add)
                    ctx.append(c)

                # ---- prenet ----
                hp1 = []
                for m4 in range(4):
                    pp1 = PP.tile([128, B], FP32, tag="ps")
                    nc.tensor.matmul(
                        pp1[:],
                        matb[:, MP_WP1A + m4 * 128:MP_WP1A + (m4 + 1) * 128],
                        cp[:, CP_XD0:CP_XD0 + B], start=True, stop=False)
                    nc.tensor.matmul(
                        pp1[:],
                        matb[0:16, MP_WP1B + m4 * 128:MP_WP1B + (m4 + 1) * 128],
                        cp[0:16, CP_XD1:CP_XD1 + B], start=False, stop=True)
                    h = PS.tile([128, B], FP32, tag=f"hp1_{m4}")
                    nc.scalar.activation(h[:], pp1[:], AF.Relu,
                                         bias=sc[:, SC_BP1 + m4:SC_BP1 + m4 + 1])
                    hp1.append(h)
                pre = []
                for m5 in range(2):
                    pp2 = PP.tile([128, B], FP32, tag="ps")
                    for k in range(4):
                        nc.tensor.matmul(
                            pp2[:],
                            matb[:, MP_WP2 + k * 256 + m5 * 128:
                                 MP_WP2 + k * 256 + (m5 + 1) * 128],
                            hp1[k][:], start=(k == 0), stop=(k == 3))
                    p = PC.tile([128, B], FP32, tag=f"pre{m5}")
                    nc.scalar.activation(p[:], pp2[:], AF.Relu,
                                         bias=sc[:, SC_BP2 + m5:SC_BP2 + m5 + 1])
                    pre.append(p)

                # ---- LSTM layers, flipped layout ----
                # g[n, :] = sum_c stat_c[k, n] * w_chunk_c[k, :]; biases ride as
                # ones-rows in the stationary + bias rows in the weight pack.
                def lstm_layer(stats, w_dram, chunks, out_tag, out_dt):
                    pg = PPG.tile([B, G3], FP32, tag="pg")
                    nch = len(chunks)
                    for ci, (row0, kc) in enumerate(chunks):
                        wt = PW.tile([kc, G3], LSTM_DT, tag="wbig",
                                     name=f"wt_{out_tag}{ci}")
                        nc.sync.dma_start(out=wt[:],
                                          in_=w_dram[row0:row0 + kc, :])
                        for nb in range(6):
                            nc.tensor.matmul(pg[:, nb * 512:(nb + 1) * 512],
                                             stats[ci],
                                             wt[:, nb * 512:(nb + 1) * 512],
                                             start=(ci == 0), stop=(ci == nch - 1))
                    # per-128-block pipelined gate math so downstream matmuls
                    # can start as soon as their stationary column is ready
                    hT = []
                    for j in range(8):
                        tg = PS.tile([B, 128], FP32, tag="tg")
                        nc.scalar.activation(tg[:],
                                             pg[:, H4 + j * 128:H4 + (j + 1) * 128],
                                             AF.Tanh)
                        si = PS.tile([B, 128], FP32, tag="si")
                        nc.scalar.activation(si[:], pg[:, j * 128:(j + 1) * 128],
                                             AF.Sigmoid)
                        cc = PS.tile([B, 128], FP32, tag="cc")
                        nc.vector.tensor_mul(cc[:], si[:], tg[:])
                        nc.scalar.activation(cc[:], cc[:], AF.Tanh)
                        so = PS.tile([B, 128], FP32, tag="so")
                        nc.scalar.activation(
                            so[:], pg[:, 2 * H4 + j * 128:2 * H4 + (j + 1) * 128],
                            AF.Sigmoid)
                        hb = PS.tile([B, 128], FP32, tag="hb")
                        nc.vector.tensor_mul(hb[:], so[:], cc[:])
                        pt = PP.tile([128, B], FP32, tag="ps")
                        nc.tensor.transpose(pt[:], hb[:], ident[:])
                        htj = PC.tile([128, B], out_dt, tag=f"{out_tag}T{j}")
                        nc.vector.tensor_copy(htj[:], pt[:])
                        hT.append(htj)
                    return hT

                def half(src_ap, p, tag):
                    t = PC.tile([p, B], LSTM_DT, tag=tag)
                    nc.vector.tensor_copy(t[:], src_ap)
                    return t

                stats0_f32 = [pre[0][:], pre[1][:], ctx[0][:], ctx[1][:],
                              ctx[2][:], ctx[3][:],
                              cp[0:SPK + 2, CP_SPKA:CP_SPKA + B]]
                if LSTM_DT is FP32:
                    stats0 = stats0_f32
                else:
                    stats0 = [half(a, a.shape[0], f"st0_{i}")[:]
                              for i, a in enumerate(stats0_f32)]
                h1T = lstm_layer(stats0, d["w0_pack"], W0_CHUNKS, "h1", LSTM_DT)
                stats1 = [t[:] for t in h1T] + [ones2[:]]
                h2T = lstm_layer(stats1, d["w1_pack"], W1_CHUNKS, "h2", FP32)

                # ---- output linear: [h2 | ctx | 1] @ [WoutT; bout] ----
                wpo = PC.tile([128, 13 * O * R], FP32, tag="wpo")
                nc.sync.dma_start(out=wpo[:], in_=d["wout_pack"][:])
                stats_o = [t[:] for t in h2T] + [c[:] for c in ctx] + [ones1[:]]
                po = PP.tile([B, O * R], FP32, tag="ps")
                for ci, st in enumerate(stats_o):
                    kc = st.shape[0]
                    mv = wpo[0:kc, ci * O * R:(ci + 1) * O * R]
                    nc.tensor.matmul(po[:], st, mv, start=(ci == 0),
                                     stop=(ci == len(stats_o) - 1))
                ob = PS.tile([B, O * R], FP32, tag="ob")
                nc.vector.tensor_copy(ob[:], po[:])

                # ---- store outputs ----
                nc.sync.dma_start(out=y_out[:], in_=ob[:])
                for k in range(4):
                    nc.sync.dma_start(out=y_ctx[k * 128:(k + 1) * 128, :],
                                      in_=ctx[k][:])

            if loop_n is None:
                body()
            else:
                with tc.For_i(0, loop_n, 1):
                    body()

    nc.finalize()
    return nc


def _prep_maps(inputs: dict) -> list:
    x = {k: np.ascontiguousarray(np.asarray(v)) for k, v in inputs.items()}
    igo = np.r_[0:H4, 2 * H4:4 * H4]

    mata = np.zeros((128, MPA_COLS), np.float32)
    wencT = x["W_enc"].T                       # [512, 256]
    for k in range(4):
        mata[:, MP_WENC + k * 256:MP_WENC + (k + 1) * 256] = \
            wencT[k * 128:(k + 1) * 128]
    mata[0:SPK, MP_WSPKR:MP_WSPKR + 256] = x["W_spkr"].T
    matb = np.zeros((128, MPB_COLS), np.float32)
    wp1T = x["W_p1"].T                         # [144, 512]
    matb[:, MP_WP1A:MP_WP1A + 512] = wp1T[0:128]
    matb[0:16, MP_WP1B:MP_WP1B + 512] = wp1T[128:144]
    wp2T = x["W_p2"].T                         # [512, 256]
    for k in range(4):
        matb[:, MP_WP2 + k * 256:MP_WP2 + (k + 1) * 256] = \
            wp2T[k * 128:(k + 1) * 128]
    wsp2T = x["W_sp2"].T                       # [256, 512]
    for k in range(2):
        matb[:, MP_WSP2 + k * 512:MP_WSP2 + (k + 1) * 512] = \
            wsp2T[k * 128:(k + 1) * 128]

    sc = np.zeros((128, SC_COLS), np.float32)
    sc[:, SC_BENC:SC_BENC + 2] = x["b_enc"].reshape(2, 128).T
    sc[:, SC_WSPEED:SC_WSPEED + 2] = x["W_speed_att"][:, 0].reshape(2, 128).T
    sc[:, SC_WPROJ:SC_WPROJ + 2] = x["W_proj"][0].reshape(2, 128).T
    sc[:, SC_BP1:SC_BP1 + 4] = x["b_p1"].reshape(4, 128).T
    sc[:, SC_BP2:SC_BP2 + 2] = x["b_p2"].reshape(2, 128).T
    sc[:, SC_WSP1:SC_WSP1 + 2] = x["W_sp1"][:, 0].reshape(2, 128).T
    sc[:, SC_BSP1:SC_BSP1 + 2] = x["b_sp1"].reshape(2, 128).T
    sc[:, SC_BSP2:SC_BSP2 + 4] = x["b_sp2"].reshape(4, 128).T
    sc[0, SC_BPROJ] = x["b_proj"][0]

    pr = np.ascontiguousarray(x["conv_prev"][:, 0, 15:15 - W:-1])  # [256, W]
    prev_pack = np.concatenate([pr[0:128], pr[128:256]],
                               axis=1).astype(np.float32)          # [128, 2W]

    w0 = np.empty((ENC + DEC + SPK + 2, G3), np.float32)
    w0[0:ENC + DEC + SPK] = x["Wih0"][igo].T
    w0[ENC + DEC + SPK] = x["bih0"][igo]
    w0[ENC + DEC + SPK + 1] = x["bhh0"][igo]
    # reorder rows so the LSTM k-chunks line up with [pre | ctx | spkr+ones]:
    # reference in_lstm = [pre(256) | context(512) | spkr(64)], and the two
    # bias rows ride with the spkr chunk -> row order stays natural; the two
    # bias rows simply extend the final 64-row chunk to 66 rows.
    w1 = np.empty((H4 + 2, G3), np.float32)
    w1[0:H4] = x["Wih1"][igo].T
    w1[H4] = x["bih1"][igo]
    w1[H4 + 1] = x["bhh1"][igo]

    wout = np.zeros((128, 13 * O * R), np.float32)
    woutT = x["W_out"].T                       # [1536, 160]
    for c in range(12):
        wout[:, c * O * R:(c + 1) * O * R] = woutT[c * 128:(c + 1) * 128]
    wout[0, 12 * O * R:13 * O * R] = x["b_out"]

    lstm_np = np.float16 if LSTM_DT == FP16 else np.float32
    shared = {
        "mat_a": mata, "mat_b": matb, "small_cols": sc,
        "prev_pack": prev_pack,
        "w0_pack": np.ascontiguousarray(w0.astype(lstm_np)),
        "w1_pack": np.ascontiguousarray(w1.astype(lstm_np)),
        "wout_pack": wout,
    }
    len1 = (np.maximum(x["lengths_enc"], 1) - 1).astype(np.float32)
    maps = []
    for c in range(N_CORES):
        nb = slice(c * B, (c + 1) * B)
        cpk = np.zeros((128, CP_COLS), np.float32)
        xeT = x["input_enc"][nb, 0:W, :].transpose(2, 0, 1).reshape(ENC, BW)
        for k in range(4):
            cpk[:, CP_XE + k * BW:CP_XE + (k + 1) * BW] = \
                xeT[k * 128:(k + 1) * 128]
        xdT = np.concatenate([x["input_dec"][nb], x["spkr_vec"][nb, 0]],
                             axis=1).T                      # [144, B]
        cpk[:, CP_XD0:CP_XD0 + B] = xdT[0:128]
        cpk[0:16, CP_XD1:CP_XD1 + B] = xdT[128:144]
        cpk[0:SPK, CP_SPKA:CP_SPKA + B] = x["spkr_vec"][nb, 0].T
        cpk[SPK:SPK + 2, CP_SPKA:CP_SPKA + B] = 1.0
        rowvec = np.concatenate([len1[nb], np.arange(W, dtype=np.float32)])
        m = dict(shared)
        m["core_pack"] = cpk
        m["rowvec"] = rowvec.reshape(1, B + W)
        m["speed_row"] = np.ascontiguousarray(x["speed"][nb].reshape(1, B))
        maps.append(m)
    return maps


def kernel(**inputs):
    if "nc" not in _CACHE:
        _CACHE["nc"] = _build_nc()
    nc = _CACHE["nc"]
    maps = _prep_maps(inputs)
    res = run_bass_kernel_spmd(nc, maps, list(range(N_CORES)))
    outs, ctxs = [], []
    for c in range(N_CORES):
        outs.append(res.results[c]["y_out"].reshape(B, R, O))
        ctxs.append(res.results[c]["y_ctx"].T[:, None, :])
    output = np.concatenate(outs, axis=0).astype(np.float32)
    context = np.concatenate(ctxs, axis=0).astype(np.float32)
    return output, context


# revision 12
# speedup vs baseline: 1.1671x; 1.1671x over previous
"""Trainium2 Bass kernel for nn_AttnDecoderRNN2 (single decoder step, eval mode).

Math: the previous-attention state is a one-hot at t=0 (null_state branch), so
the windowed mask keeps only positions t in [0, ATT_RANGE), and the global max
subtraction cancels exactly in the L1 normalization.  The [N, T, ATT]
attention-bias tensor therefore only matters on a 10-wide window.  The zero
initial LSTM state kills the Whh terms and the f gate, so only the i/g/o
columns of Wih0/Wih1 are shipped.

Layout: the big LSTM/output matmuls run "flipped" — activations are the
stationary operand ([K, batch] column tiles), weight chunks are the moving
operand streamed straight from HBM — so the PE never pays fp32 LDWEIGHTS on
the 24 MB of weights and compute overlaps the weight DMA chunk-by-chunk.
Biases ride along as ones-rows appended to the stationary operand with the
bias vectors appended as extra weight rows.

Sharding: pure data-parallel over batch N=64 across 8 cores (8 batches/core),
weights replicated, each packed host-side into a handful of large DMA-friendly
tensors.
"""

import numpy as np

import concourse.bass as bass
import concourse.mybir as mybir
import concourse.tile as tile
from concourse import bacc
from concourse.bass_utils import run_bass_kernel_spmd
from concourse.masks import make_identity

AF = mybir.ActivationFunctionType
ALU = mybir.AluOpType
FP32 = mybir.dt.float32
FP16 = mybir.dt.float16
LSTM_DT = FP16          # dtype of the streamed LSTM weight chunks + stationaries

N_CORES = 8
N = 64
ENC, ATT, DEC, SPK, O, R = 512, 256, 256, 64, 80, 2
H4 = 4 * DEC                 # 1024
G3 = 3 * H4                  # 3072 gates kept (i, g, o; f is dead)
B = N // N_CORES             # batches per core = 8
W = 10                       # attention window = ATT_RANGE
BW = B * W                   # 80

# mat_a column offsets (attention weights, loaded first)
MP_WENC = 0            # 4 blocks x 256
MP_WSPKR = 1024        # [0:64] x 256
MPA_COLS = 1280
# mat_b column offsets (prenet / speed-proj weights)
MP_WP1A = 0            # [128] x 512
MP_WP1B = 512          # [0:16] x 512
MP_WP2 = 1024          # 4 blocks x 256
MP_WSP2 = 2048         # 2 blocks x 512
MPB_COLS = 3072

# small_cols column offsets
SC_BENC, SC_WSPEED, SC_WPROJ, SC_BP1, SC_BP2, SC_WSP1, SC_BSP1, SC_BSP2, \
    SC_BPROJ = 0, 2, 4, 6, 10, 12, 14, 16, 20
SC_COLS = 21

# core_pack column offsets
CP_XE = 0              # 4 blocks x 80
CP_XD0 = 320           # [128] x 8
CP_XD1 = 328           # [0:16] x 8
CP_SPKA = 336          # [0:66] x 8 (rows 64:66 = 1.0 bias-ones)
CP_COLS = 344

_CACHE: dict = {}


def _build_nc(loop_n=None):
    nc = bacc.Bacc(None)

    def inp(name, shape):
        return nc.dram_tensor(name, list(shape), FP32, kind="ExternalInput")

    d = {
        "core_pack": inp("core_pack", [128, CP_COLS]),
        "rowvec": inp("rowvec", [1, B + W]),
        "speed_row": inp("speed_row", [1, B]),
        "mat_a": inp("mat_a", [128, MPA_COLS]),
        "mat_b": inp("mat_b", [128, MPB_COLS]),
        "small_cols": inp("small_cols", [128, SC_COLS]),
        "prev_pack": inp("prev_pack", [128, 2 * W]),
        "w0_pack": nc.dram_tensor("w0_pack", [ENC + DEC + SPK + 2, G3], LSTM_DT,
                                  kind="ExternalInput"),
        "w1_pack": nc.dram_tensor("w1_pack", [H4 + 2, G3], LSTM_DT,
                                  kind="ExternalInput"),
        "wout_pack": inp("wout_pack", [128, 13 * O * R]),
    }
    y_out = nc.dram_tensor("y_out", [B, O * R], FP32, kind="ExternalOutput")
    y_ctx = nc.dram_tensor("y_ctx", [ENC, B], FP32, kind="ExternalOutput")

    W0_CHUNKS = [(k * 128, 128) for k in range(6)] + [(768, 66)]
    W1_CHUNKS = [(k * 128, 128) for k in range(8)] + [(1024, 2)]

    with tile.TileContext(nc) as tc:
        with (
            tc.tile_pool(name="const", bufs=1 if loop_n is None else 2) as PC,
            tc.tile_pool(name="wbig", bufs=9) as PW,
            tc.tile_pool(name="scr", bufs=2) as PS,
            tc.tile_pool(name="ps", bufs=2, space="PSUM") as PP,
            tc.tile_pool(name="pg", bufs=1, space="PSUM") as PPG,
        ):
            def body():
                # ---- bulk constant loads ----
                cp = PC.tile([128, CP_COLS], FP32, tag="cp")
                nc.sync.dma_start(out=cp[:], in_=d["core_pack"][:])
                rv = PC.tile([1, B + W], FP32, tag="rv")
                nc.sync.dma_start(out=rv[:], in_=d["rowvec"][:])
                speed_bc = PC.tile([128, B], FP32, tag="speed_bc")
                nc.sync.dma_start(out=speed_bc[:],
                                  in_=d["speed_row"][:].to_broadcast([128, B]))
                mata = PC.tile([128, MPA_COLS], FP32, tag="mata")
                nc.sync.dma_start(out=mata[:], in_=d["mat_a"][:])
                sc = PC.tile([128, SC_COLS], FP32, tag="sc")
                nc.sync.dma_start(out=sc[:], in_=d["small_cols"][:])
                prev = PC.tile([128, 2 * W], FP32, tag="prev")
                nc.sync.dma_start(out=prev[:], in_=d["prev_pack"][:])
                matb = PC.tile([128, MPB_COLS], FP32, tag="matb")
                nc.sync.dma_start(out=matb[:], in_=d["mat_b"][:])
                ident = PC.tile([B, B], FP32, tag="ident")
                make_identity(nc, ident[:])
                ones1 = PC.tile([1, B], FP32, tag="ones1")
                nc.vector.memset(ones1[:], 1.0)
                ones2 = PC.tile([2, B], LSTM_DT, tag="ones2")
                nc.vector.memset(ones2[:], 1.0)
                onesrep = PC.tile([1, 128], FP32, tag="onesrep")
                nc.vector.memset(onesrep[:], 1.0)

                xe = [cp[:, CP_XE + k * BW:CP_XE + (k + 1) * BW] for k in range(4)]
                len1 = rv[:, 0:B]
                t_row = rv[:, B:B + W]

                def softsign(tagbase, src_ap, p, f):
                    ax = PS.tile([p, f], FP32, tag=tagbase + "_a")
                    nc.scalar.activation(ax[:], src_ap, AF.Abs)
                    den = PS.tile([p, f], FP32, tag=tagbase + "_d")
                    nc.vector.tensor_scalar_add(den[:], ax[:], 1.0)
                    rec = PS.tile([p, f], FP32, tag=tagbase + "_r")
                    nc.vector.reciprocal(rec[:], den[:])
                    out = PS.tile([p, f], FP32, tag=tagbase)
                    nc.vector.tensor_mul(out[:], src_ap, rec[:])
                    return out

                # ---- attention bias from encoder window ----
                ab = []
                for m in range(2):
                    pe = PP.tile([128, BW], FP32, tag="ps")
                    for k in range(4):
                        nc.tensor.matmul(
                            pe[:],
                            mata[:, MP_WENC + k * 256 + m * 128:
                                 MP_WENC + k * 256 + (m + 1) * 128],
                            xe[k], start=(k == 0), stop=(k == 3))
                    eb = PS.tile([128, BW], FP32, tag=f"eb{m}")
                    nc.vector.tensor_scalar_add(eb[:], pe[:],
                                                sc[:, SC_BENC + m:SC_BENC + m + 1])
                    ab.append(softsign(f"ab{m}", eb[:], 128, BW))

                # ---- speaker bias ----
                sb = []
                for m in range(2):
                    pk = PP.tile([128, B], FP32, tag="ps")
                    nc.tensor.matmul(
                        pk[:],
                        mata[0:SPK, MP_WSPKR + m * 128:MP_WSPKR + (m + 1) * 128],
                        cp[0:SPK, CP_SPKA:CP_SPKA + B], start=True, stop=True)
                    sb.append(softsign(f"sb{m}", pk[:], 128, B))

                # ---- bias_an[a, n] = spkr_bias + speed * wspeed ----
                bias_an = []
                for m in range(2):
                    scv = PS.tile([128, B], FP32, tag=f"spc{m}")
                    nc.vector.tensor_scalar_mul(
                        scv[:], speed_bc[:], sc[:, SC_WSPEED + m:SC_WSPEED + m + 1])
                    bn = PS.tile([128, B], FP32, tag=f"ban{m}")
                    nc.vector.tensor_add(bn[:], sb[m][:], scv[:])
                    bias_an.append(bn)

                # ---- e = ab + bias_an (bcast t) + prev (bcast n); tanh ----
                th = []
                for m in range(2):
                    e3 = PS.tile([128, B, W], FP32, tag=f"e3{m}")
                    ab3 = ab[m][:].rearrange("p (n w) -> p n w", w=W)
                    bn3 = bias_an[m][:].unsqueeze(2).to_broadcast([128, B, W])
                    nc.vector.tensor_add(e3[:], ab3, bn3)
                    pv3 = prev[:, m * W:(m + 1) * W].unsqueeze(1) \
                        .to_broadcast([128, B, W])
                    nc.vector.tensor_add(e3[:], e3[:], pv3)
                    t3 = PS.tile([128, BW], FP32, tag=f"th{m}")
                    nc.scalar.activation(t3[:],
                                         e3[:].rearrange("p n w -> p (n w)"),
                                         AF.Tanh)
                    th.append(t3)

                # ---- logit = tanh(e) . wproj + bproj ----
                pl = PP.tile([1, BW], FP32, tag="ps")
                for m in range(2):
                    nc.tensor.matmul(pl[:], sc[:, SC_WPROJ + m:SC_WPROJ + m + 1],
                                     th[m][:], start=(m == 0), stop=(m == 1))
                logit = PS.tile([1, BW], FP32, tag="logit")
                nc.vector.tensor_scalar_add(logit[:], pl[:],
                                            sc[0:1, SC_BPROJ:SC_BPROJ + 1])

                # ---- windowed masked softmax (L1-normalized) ----
                lg3 = logit[:].rearrange("p (n w) -> p n w", w=W)
                mx = PS.tile([1, B], FP32, tag="mx")
                nc.vector.tensor_reduce(mx[:], lg3, mybir.AxisListType.X, ALU.max)
                sh = PS.tile([1, B, W], FP32, tag="sh")
                nc.vector.tensor_tensor(sh[:], lg3,
                                        mx[:].unsqueeze(2).to_broadcast([1, B, W]),
                                        ALU.subtract)
                ex = PS.tile([1, B, W], FP32, tag="ex")
                nc.scalar.activation(ex[:], sh[:], AF.Exp)
                mask = PS.tile([1, B, W], FP32, tag="mask")
                nc.vector.tensor_tensor(
                    mask[:], t_row.unsqueeze(1).to_broadcast([1, B, W]),
                    len1.unsqueeze(2).to_broadcast([1, B, W]), ALU.is_le)
                wm = PS.tile([1, B, W], FP32, tag="wm")
                nc.vector.tensor_mul(wm[:], ex[:], mask[:])
                den = PS.tile([1, B], FP32, tag="den")
                nc.vector.tensor_reduce(den[:], wm[:], mybir.AxisListType.X, ALU.add)
                nc.vector.tensor_scalar_max(den[:], den[:], 1e-12)
                rec = PS.tile([1, B], FP32, tag="recd")
                nc.vector.reciprocal(rec[:], den[:])
                att = PS.tile([1, B, W], FP32, tag="att")
                nc.vector.tensor_tensor(att[:], wm[:],
                                        rec[:].unsqueeze(2).to_broadcast([1, B, W]),
                                        ALU.mult)

                # ---- replicate att across 128 partitions via K=1 matmul ----
                pr = PP.tile([128, BW], FP32, tag="ps")
                nc.tensor.matmul(pr[:], onesrep[:],
                                 att[:].rearrange("p n w -> p (n w)"),
                                 start=True, stop=True)
                att_rep = PS.tile([128, BW], FP32, tag="attrep")
                nc.vector.tensor_copy(att_rep[:], pr[:])

                # ---- speed projection sp = tanh(relu(speed*w+b) @ Wsp2T + b2) ----
                r1 = []
                for m in range(2):
                    r = PS.tile([128, B], FP32, tag=f"r1{m}")
                    nc.vector.tensor_scalar(r[:], speed_bc[:],
                                            sc[:, SC_WSP1 + m:SC_WSP1 + m + 1],
                                            sc[:, SC_BSP1 + m:SC_BSP1 + m + 1],
                                            ALU.mult, ALU.add)
                    nc.scalar.activation(r[:], r[:], AF.Relu)
                    r1.append(r)
                sp_sb = []
                for m3 in range(4):
                    psp = PP.tile([128, B], FP32, tag="ps")
                    for k in range(2):
                        nc.tensor.matmul(
                            psp[:],
                            matb[:, MP_WSP2 + k * 512 + m3 * 128:
                                 MP_WSP2 + k * 512 + (m3 + 1) * 128],
                            r1[k][:], start=(k == 0), stop=(k == 1))
                    s = PS.tile([128, B], FP32, tag=f"sp{m3}")
                    nc.scalar.activation(s[:], psp[:], AF.Tanh,
                                         bias=sc[:, SC_BSP2 + m3:SC_BSP2 + m3 + 1])
                    sp_sb.append(s)

                # ---- context[e, n] = sum_t att[n,t] * (xe + sp)[e, n, t] ----
                ctx = []
                for k in range(4):
                    em = PS.tile([128, B, W], FP32, tag=f"em{k}")
                    nc.vector.tensor_add(
                        em[:], xe[k].rearrange("p (n w) -> p n w", w=W),
                        sp_sb[k][:].unsqueeze(2).to_broadcast([128, B, W]))
                    nc.vector.tensor_mul(
                        em[:], em[:], att_rep[:].rearrange("p (n w) -> p n w", w=W))
                    c = PC.tile([128, B], FP32, tag=f"ctx{k}")
                    nc.vector.tensor_reduce(c[:], em[:], mybir.AxisListType.X,
                                            ALU.add)
                    ctx.append(c)

                # ---- prenet ----
                hp1 = []
                for m4 in range(4):
                    pp1 = PP.tile([128, B], FP32, tag="ps")
                    nc.tensor.matmul(
                        pp1[:],
                        matb[:, MP_WP1A + m4 * 128:MP_WP1A + (m4 + 1) * 128],
                        cp[:, CP_XD0:CP_XD0 + B], start=True, stop=False)
                    nc.tensor.matmul(
                        pp1[:],
                        matb[0:16, MP_WP1B + m4 * 128:MP_WP1B + (m4 + 1) * 128],
                        cp[0:16, CP_XD1:CP_XD1 + B], start=False, stop=True)
                    h = PS.tile([128, B], FP32, tag=f"hp1_{m4}")
                    nc.scalar.activation(h[:], pp1[:], AF.Relu,
                                         bias=sc[:, SC_BP1 + m4:SC_BP1 + m4 + 1])
                    hp1.append(h)
                pre = []
                for m5 in range(2):
                    pp2 = PP.tile([128, B], FP32, tag="ps")
                    for k in range(4):
                        nc.tensor.matmul(
                            pp2[:],
                            matb[:, MP_WP2 + k * 256 + m5 * 128:
                                 MP_WP2 + k * 256 + (m5 + 1) * 128],
                            hp1[k][:], start=(k == 0), stop=(k == 3))
                    p = PC.tile([128, B], FP32, tag=f"pre{m5}")
                    nc.scalar.activation(p[:], pp2[:], AF.Relu,
                                         bias=sc[:, SC_BP2 + m5:SC_BP2 + m5 + 1])
                    pre.append(p)

                # ---- LSTM layers, flipped layout ----
                # g[n, :] = sum_c stat_c[k, n] * w_chunk_c[k, :]; biases ride as
                # ones-rows in the stationary + bias rows in the weight pack.
                def lstm_layer(stats, w_dram, chunks, out_tag, out_dt):
                    pg = PPG.tile([B, G3], FP32, tag="pg")
                    nch = len(chunks)
                    for ci, (row0, kc) in enumerate(chunks):
                        wt = PW.tile([kc, G3], LSTM_DT, tag="wbig",
                                     name=f"wt_{out_tag}{ci}")
                        nc.sync.dma_start(out=wt[:],
                                          in_=w_dram[row0:row0 + kc, :])
                        for nb in range(6):
                            nc.tensor.matmul(pg[:, nb * 512:(nb + 1) * 512],
                                             stats[ci],
                                             wt[:, nb * 512:(nb + 1) * 512],
                                             start=(ci == 0), stop=(ci == nch - 1))
                    tg = PS.tile([B, H4], FP32, tag="tg")
                    nc.scalar.activation(tg[:], pg[:, H4:2 * H4], AF.Tanh)
                    si = PS.tile([B, H4], FP32, tag="si")
                    nc.scalar.activation(si[:], pg[:, 0:H4], AF.Sigmoid)
                    cc = PS.tile([B, H4], FP32, tag="cc")
                    nc.vector.tensor_mul(cc[:], si[:], tg[:])
                    nc.scalar.activation(cc[:], cc[:], AF.Tanh)
                    so = PS.tile([B, H4], FP32, tag="so")
                    nc.scalar.activation(so[:], pg[:, 2 * H4:3 * H4], AF.Sigmoid)
                    h = PS.tile([B, H4], FP32, tag=out_tag)
                    nc.vector.tensor_mul(h[:], so[:], cc[:])
                    hT = []
                    for j in range(8):
                        pt = PP.tile([128, B], FP32, tag="ps")
                        nc.tensor.transpose(pt[:], h[:, j * 128:(j + 1) * 128],
                                            ident[:])
                        htj = PC.tile([128, B], out_dt, tag=f"{out_tag}T{j}")
                        nc.vector.tensor_copy(htj[:], pt[:])
                        hT.append(htj)
                    return hT

                def half(src_ap, p, tag):
                    t = PC.tile([p, B], LSTM_DT, tag=tag)
                    nc.vector.tensor_copy(t[:], src_ap)
                    return t

                stats0_f32 = [pre[0][:], pre[1][:], ctx[0][:], ctx[1][:],
                              ctx[2][:], ctx[3][:],
                              cp[0:SPK + 2, CP_SPKA:CP_SPKA + B]]
                if LSTM_DT is FP32:
                    stats0 = stats0_f32
                else:
                    stats0 = [half(a, a.shape[0], f"st0_{i}")[:]
                              for i, a in enumerate(stats0_f32)]
                h1T = lstm_layer(stats0, d["w0_pack"], W0_CHUNKS, "h1", LSTM_DT)
                stats1 = [t[:] for t in h1T] + [ones2[:]]
                h2T = lstm_layer(stats1, d["w1_pack"], W1_CHUNKS, "h2", FP32)

                # ---- output linear: [h2 | ctx | 1] @ [WoutT; bout] ----
                wpo = PC.tile([128, 13 * O * R], FP32, tag="wpo")
                nc.sync.dma_start(out=wpo[:], in_=d["wout_pack"][:])
                stats_o = [t[:] for t in h2T] + [c[:] for c in ctx] + [ones1[:]]
                po = PP.tile([B, O * R], FP32, tag="ps")
                for ci, st in enumerate(stats_o):
                    kc = st.shape[0]
                    mv = wpo[0:kc, ci * O * R:(ci + 1) * O * R]
                    nc.tensor.matmul(po[:], st, mv, start=(ci == 0),
                                     stop=(ci == len(stats_o) - 1))
                ob = PS.tile([B, O * R], FP32, tag="ob")
                nc.vector.tensor_copy(ob[:], po[:])

                # ---- store outputs ----
                nc.sync.dma_start(out=y_out[:], in_=ob[:])
                for k in range(4):
                    nc.sync.dma_start(out=y_ctx[k * 128:(k + 1) * 128, :],
                                      in_=ctx[k][:])

            if loop_n is None:
                body()
            else:
                with tc.For_i(0, loop_n, 1):
                    body()

    nc.finalize()
    return nc


def _prep_maps(inputs: dict) -> list:
    x = {k: np.ascontiguousarray(np.asarray(v)) for k, v in inputs.items()}
    igo = np.r_[0:H4, 2 * H4:4 * H4]

    mata = np.zeros((128, MPA_COLS), np.float32)
    wencT = x["W_enc"].T                       # [512, 256]
    for k in range(4):
        mata[:, MP_WENC + k * 256:MP_WENC + (k + 1) * 256] = \
            wencT[k * 128:(k + 1) * 128]
    mata[0:SPK, MP_WSPKR:MP_WSPKR + 256] = x["W_spkr"].T
    matb = np.zeros((128, MPB_COLS), np.float32)
    wp1T = x["W_p1"].T                         # [144, 512]
    matb[:, MP_WP1A:MP_WP1A + 512] = wp1T[0:128]
    matb[0:16, MP_WP1B:MP_WP1B + 512] = wp1T[128:144]
    wp2T = x["W_p2"].T                         # [512, 256]
    for k in range(4):
        matb[:, MP_WP2 + k * 256:MP_WP2 + (k + 1) * 256] = \
            wp2T[k * 128:(k + 1) * 128]
    wsp2T = x["W_sp2"].T                       # [256, 512]
    for k in range(2):
        matb[:, MP_WSP2 + k * 512:MP_WSP2 + (k + 1) * 512] = \
            wsp2T[k * 128:(k + 1) * 128]

    sc = np.zeros((128, SC_COLS), np.float32)
    sc[:, SC_BENC:SC_BENC + 2] = x["b_enc"].reshape(2, 128).T
    sc[:, SC_WSPEED:SC_WSPEED + 2] = x["W_speed_att"][:, 0].reshape(2, 128).T
    sc[:, SC_WPROJ:SC_WPROJ + 2] = x["W_proj"][0].reshape(2, 128).T
    sc[:, SC_BP1:SC_BP1 + 4] = x["b_p1"].reshape(4, 128).T
    sc[:, SC_BP2:SC_BP2 + 2] = x["b_p2"].reshape(2, 128).T
    sc[:, SC_WSP1:SC_WSP1 + 2] = x["W_sp1"][:, 0].reshape(2, 128).T
    sc[:, SC_BSP1:SC_BSP1 + 2] = x["b_sp1"].reshape(2, 128).T
    sc[:, SC_BSP2:SC_BSP2 + 4] = x["b_sp2"].reshape(4, 128).T
    sc[0, SC_BPROJ] = x["b_proj"][0]

    pr = np.ascontiguousarray(x["conv_prev"][:, 0, 15:15 - W:-1])  # [256, W]
    prev_pack = np.concatenate([pr[0:128], pr[128:256]],
                               axis=1).astype(np.float32)          # [128, 2W]

    w0 = np.empty((ENC + DEC + SPK + 2, G3), np.float32)
    w0[0:ENC + DEC + SPK] = x["Wih0"][igo].T
    w0[ENC + DEC + SPK] = x["bih0"][igo]
    w0[ENC + DEC + SPK + 1] = x["bhh0"][igo]
    # reorder rows so the LSTM k-chunks line up with [pre | ctx | spkr+ones]:
    # reference in_lstm = [pre(256) | context(512) | spkr(64)], and the two
    # bias rows ride with the spkr chunk -> row order stays natural; the two
    # bias rows simply extend the final 64-row chunk to 66 rows.
    w1 = np.empty((H4 + 2, G3), np.float32)
    w1[0:H4] = x["Wih1"][igo].T
    w1[H4] = x["bih1"][igo]
    w1[H4 + 1] = x["bhh1"][igo]

    wout = np.zeros((128, 13 * O * R), np.float32)
    woutT = x["W_out"].T                       # [1536, 160]
    for c in range(12):
        wout[:, c * O * R:(c + 1) * O * R] = woutT[c * 128:(c + 1) * 128]
    wout[0, 12 * O * R:13 * O * R] = x["b_out"]

    lstm_np = np.float16 if LSTM_DT == FP16 else np.float32
    shared = {
        "mat_a": mata, "mat_b": matb, "small_cols": sc,
        "prev_pack": prev_pack,
        "w0_pack": np.ascontiguousarray(w0.astype(lstm_np)),
        "w1_pack": np.ascontiguousarray(w1.astype(lstm_np)),
        "wout_pack": wout,
    }
    len1 = (np.maximum(x["lengths_enc"], 1) - 1).astype(np.float32)
    maps = []
    for c in range(N_CORES):
        nb = slice(c * B, (c + 1) * B)
        cpk = np.zeros((128, CP_COLS), np.float32)
        xeT = x["input_enc"][nb, 0:W, :].transpose(2, 0, 1).reshape(ENC, BW)
        for k in range(4):
            cpk[:, CP_XE + k * BW:CP_XE + (k + 1) * BW] = \
                xeT[k * 128:(k + 1) * 128]
        xdT = np.concatenate([x["input_dec"][nb], x["spkr_vec"][nb, 0]],
                             axis=1).T                      # [144, B]
        cpk[:, CP_XD0:CP_XD0 + B] = xdT[0:128]
        cpk[0:16, CP_XD1:CP_XD1 + B] = xdT[128:144]
        cpk[0:SPK, CP_SPKA:CP_SPKA + B] = x["spkr_vec"][nb, 0].T
        cpk[SPK:SPK + 2, CP_SPKA:CP_SPKA + B] = 1.0
        rowvec = np.concatenate([len1[nb], np.arange(W, dtype=np.float32)])
        m = dict(shared)
        m["core_pack"] = cpk
        m["rowvec"] = rowvec.reshape(1, B + W)
        m["speed_row"] = np.ascontiguousarray(x["speed"][nb].reshape(1, B))
        maps.append(m)
    return maps


def kernel(**inputs):
    if "nc" not in _CACHE:
        _CACHE["nc"] = _build_nc()
    nc = _CACHE["nc"]
    maps = _prep_maps(inputs)
    res = run_bass_kernel_spmd(nc, maps, list(range(N_CORES)))
    outs, ctxs = [], []
    for c in range(N_CORES):
        outs.append(res.results[c]["y_out"].reshape(B, R, O))
        ctxs.append(res.results[c]["y_ctx"].T[:, None, :])
    output = np.concatenate(outs, axis=0).astype(np.float32)
    context = np.concatenate(ctxs, axis=0).astype(np.float32)
    return output, context
